# revision 2
# baseline (speedup 1.0000x reference)
"""Trainium2 Bass kernel for CCHead — Winograd F(2x2,3x3) version.

Self-contained: kernel(**inputs) takes the full unsharded inputs
(x[8, 2048, 64, 64] + weights), shards batch across 8 NeuronCores
(1 image per core, all params replicated), and returns the full
output [8, 104, 64, 64] float32.

The three 3x3 convs (conva 2048->512, convb 512->512, bott 2560->512)
run as Winograd F(2x2,3x3): 2.25x fewer PE MACs. U = GwG^T computed on
host (bf16, per-(uv,ci-block,co-block) [128,128] tiles streamed from
DRAM per stripe); V = B^T d B computed on DVE/GpSimd in bf16 from the
padded flat activation layout; 16 uv GEMMs accumulate over ci blocks
into PSUM [128,2,4,256] tiles; inverse A^T M A runs as bf16 Z-partials
(vector engine) + f32 Y combine + scalar bias+ReLU back into the same
flat layout the criss-cross attention code reads. Attention unchanged.
"""
import contextlib
import numpy as np
import ml_dtypes
import concourse.bass as bass
import concourse.tile as tile
from concourse import bacc, mybir

f32 = mybir.dt.float32
bf16 = mybir.dt.bfloat16
AF = mybir.ActivationFunctionType
AX = mybir.AxisListType
BF_NP = ml_dtypes.bfloat16

S = 65
NR = 67
FLAT = NR * S + 2          # 4357 (src tiles: lead pad + 67 padded rows + slack)
IMG0 = 1 + S               # flat offset of image row 0, col 0
XFLAT = 74 * S + 2         # padded x in DRAM
WLEN = 18 * S + 2          # conv window: 18 padded row slots + slack
WLEN_W = 3 * S + 16 * S + 4  # window tile width (AP bound slack for row bands)
X_DEV_SHAPE = (16, 128, XFLAT)
QK_TILES = [(i * 512, 512) for i in range(8)] + [(4096, 64)]

# Winograd F(2x2,3x3) transforms
G_MAT = np.array([[1, 0, 0], [.5, .5, .5], [.5, -.5, .5], [0, 0, 1]], np.float32)
# T-row combos per uv-half jh: (rowoff_a, rowoff_b, is_add); rows rel. to 2*tr
J_ROWS = [[(-1, 1, False), (0, 1, True)], [(1, 0, False), (0, 2, False)]]


def pad_x_host(x_core):
    """[2048, 64, 64] f32 -> [16, 128, XFLAT] bf16 padded flat."""
    xb = x_core.reshape(16, 128, 64, 64).astype(BF_NP)
    arr = np.zeros((16, 128, XFLAT), BF_NP)
    arr[:, :, 1:1 + NR * S].reshape(16, 128, NR, S)[:, :, 1:65, 0:64] = xb
    return arr


def make_u(w):
    """folded [co, ci, 3, 3] f32 -> U [2, 4, nci, 128, 8, 128] bf16."""
    co, ci = w.shape[:2]
    u = np.einsum('jr,us,oirs->juoi', G_MAT, G_MAT, w.astype(np.float32))
    u = u.reshape(2, 2, 4, co // 128, 128, ci // 128, 128)  # jh jl u cob col cb cil
    u = u.transpose(0, 3, 5, 6, 1, 2, 4)  # jh cob cb cil jl u col
    return np.ascontiguousarray(
        u.reshape(2, co // 128, ci // 128, 128, 8, 128).astype(BF_NP))


def host_prep(inputs):
    f = np.float32

    def fold(w, g, b, m, v):
        s = (g / np.sqrt(v + 1e-5)).astype(f)
        return (w * s[:, None, None, None]).astype(f), (b - m * s).astype(f)

    def t1x1(w):  # [co, ci, 1, 1] -> [nci, 128, co] bf16
        co, ci = w.shape[:2]
        return np.ascontiguousarray(
            w.reshape(co, ci).T.reshape(ci // 128, 128, co).astype(BF_NP))

    wa, ba = fold(inputs['conva_w'], inputs['conva_g'], inputs['conva_b'],
                  inputs['conva_m'], inputs['conva_v'])
    wb, bb = fold(inputs['convb_w'], inputs['convb_g'], inputs['convb_b'],
                  inputs['convb_m'], inputs['convb_v'])
    wt, bt = fold(inputs['bott_w'], inputs['bott_g'], inputs['bott_b'],
                  inputs['bott_m'], inputs['bott_v'])
    gamma = float(np.asarray(inputs['cc_gamma']).reshape(-1)[0])
    mask = np.zeros((64, 64), f)
    np.fill_diagonal(mask, -1e30)
    mask4 = np.ascontiguousarray(
        np.broadcast_to(mask[:, None, :], (64, 4, 64)).astype(f))
    dev = {
        'ua': make_u(wa), 'ba': ba.reshape(4, 128, 1),
        'ub': make_u(wb), 'bb': bb.reshape(4, 128, 1),
        'ut': make_u(wt), 'bt': bt.reshape(4, 128, 1),
        'wc': t1x1(inputs['cls_w']),
        'bc': inputs['cls_b'].astype(f).reshape(104, 1),
        'wq': t1x1(inputs['q_w']), 'bq': inputs['q_b'].astype(f).reshape(64, 1),
        'wk': t1x1(inputs['k_w']), 'bk': inputs['k_b'].astype(f).reshape(64, 1),
        'wv': t1x1(inputs['v_w']),
        'gvb': (gamma * inputs['v_b']).astype(f).reshape(4, 128, 1),
        'mask4': mask4,
        'ident': np.eye(64, dtype=BF_NP),
        'identf': np.eye(64, dtype=f),
    }
    return dev, gamma


INPUT_SPECS = [
    ('ua', [2, 4, 16, 128, 8, 128], bf16), ('ba', [4, 128, 1], f32),
    ('ub', [2, 4, 4, 128, 8, 128], bf16), ('bb', [4, 128, 1], f32),
    ('ut', [2, 4, 20, 128, 8, 128], bf16), ('bt', [4, 128, 1], f32),
    ('wc', [4, 128, 104], bf16), ('bc', [104, 1], f32),
    ('wq', [4, 128, 64], bf16), ('bq', [64, 1], f32),
    ('wk', [4, 128, 64], bf16), ('bk', [64, 1], f32),
    ('wv', [4, 128, 512], bf16),
    ('gvb', [4, 128, 1], f32),
    ('mask4', [64, 4, 64], f32),
    ('ident', [64, 64], bf16),
    ('identf', [64, 64], f32),
]


def build(gamma, n_reps=1):
    nc = bacc.Bacc("TRN2", num_devices=8)
    t = {'x': nc.dram_tensor("x", list(X_DEV_SHAPE), bf16, kind="ExternalInput")}
    for nm, shape, dt in INPUT_SPECS:
        t[nm] = nc.dram_tensor(nm, shape, dt, kind="ExternalInput")
    y = nc.dram_tensor("y", [104, 64, 64], f32, kind="ExternalOutput")
    with tile.TileContext(nc) as tc:
        _build_body(tc, t, y, gamma, n_reps)
    nc.compile()
    return nc


def _rows(flat_tile):
    """[128, FLAT] -> padded row view [128, 67, 65] (skips lead pad elem)."""
    return flat_tile[:, 1:1 + NR * S].rearrange("p (r c) -> p r c", c=S)


def _band(sflat, base_slot, s, rowoff):
    """rows 2*(8s+k)+rowoff for k=0..7, cols c'=0..65 -> [128, 8, 66] AP."""
    off0 = base_slot(16 * s + rowoff) * S
    return sflat[:, off0:off0 + 16 * S].rearrange(
        "q (k c) -> q k c", c=2 * S)[:, :, 0:66]


def _build_body(tc, t, y, gamma, n_reps):
    nc = tc.nc
    with contextlib.ExitStack() as est:
        cp = est.enter_context(tc.tile_pool(name="const", bufs=1))

        def cload(nm, shape, dt):
            tl = cp.tile(shape, dt, tag=nm, name=nm)
            nc.sync.dma_start(tl[:], t[nm][:])
            return tl

        def load_blocks(nm, n, shape, dt=f32):
            out = []
            for i in range(n):
                tl = cp.tile(shape, dt, tag=f"{nm}{i}", name=f"{nm}{i}")
                nc.sync.dma_start(tl[:], t[nm][i])
                out.append(tl)
            return out

        C = dict(nc=nc, tc=tc, t=t, y=y, gamma=gamma,
                 bias_a=load_blocks('ba', 4, [128, 1]))

        def load_late():
            C.update(ident=cload('ident', [64, 64], bf16),
                     identf=cload('identf', [64, 64], f32),
                     mask4=cload('mask4', [64, 4, 64], f32),
                     bq=cload('bq', [64, 1], f32),
                     bk=cload('bk', [64, 1], f32),
                     bc=cload('bc', [104, 1], f32),
                     bias_b=load_blocks('bb', 4, [128, 1]),
                     bias_t=load_blocks('bt', 4, [128, 1]),
                     gvb=load_blocks('gvb', 4, [128, 1]),
                     wq=load_blocks('wq', 4, [128, 64], bf16),
                     wk=load_blocks('wk', 4, [128, 64], bf16),
                     wv=load_blocks('wv', 4, [128, 512], bf16),
                     wc=load_blocks('wc', 4, [128, 104], bf16))
        C['load_late'] = load_late

        for _ in range(n_reps):
            _network(C, est)


def _network(C, est):
    nc, tc, t = C['nc'], C['tc'], C['t']
    sbp = est.enter_context(tc.tile_pool(name="sbp", bufs=1))
    srcB = [sbp.tile([128, FLAT], bf16, tag=f"sb{i}", name=f"sb{i}")
            for i in range(4)]
    for blk in srcB:
        nc.any.memset(blk[:], 0.0)
    with tc.tile_pool(name="sap", bufs=1) as sap:
        srcA = [sap.tile([128, FLAT], bf16, tag=f"sa{i}", name=f"sa{i}")
                for i in range(4)]
        for blk in srcA:
            nc.any.memset(blk[:], 0.0)
        # conva: x -> srcA (Winograd)
        _wino_conv(C, 'x', 16, t['ua'], C['bias_a'], dst_sbuf=srcA)
        C.pop('load_late')()
        # CCA 1: srcA -> srcB;  CCA 2: srcB -> srcA
        _cca(C, srcA, srcB)
        _cca(C, srcB, srcA)
        # convb: srcA -> srcB (Winograd)
        _wino_conv(C, srcA, 4, t['ub'], C['bias_b'], dst_sbuf=srcB)
    # bott: x (16cb) + srcB (4cb) -> ot (Winograd), then cls -> y
    with contextlib.ExitStack() as eso:
        otp = eso.enter_context(tc.tile_pool(name="otp", bufs=1))
        ot = [otp.tile([128, 64, 64], bf16, tag=f"ot{i}", name=f"ot{i}")
              for i in range(4)]
        _wino_conv(C, ('mix', srcB), 20, t['ut'], C['bias_t'], dst_flat=ot)
        cop = eso.enter_context(tc.tile_pool(name="cop", bufs=1))
        cpp = eso.enter_context(tc.tile_pool(name="cpp", bufs=2, space="PSUM"))
        out_sb = cop.tile([104, 64, 64], f32)
        oflat = out_sb[:].rearrange("p r c -> p (r c)")
        for off, n in [(i * 512, 512) for i in range(8)]:
            ps = cpp.tile([104, 512], f32, tag="clsps")
            for cb in range(4):
                rhs = ot[cb][:].rearrange("p r c -> p (r c)")[:, off:off + n]
                nc.tensor.matmul(ps[:, 0:n], C['wc'][cb][:], rhs,
                                 start=(cb == 0), stop=(cb == 3))
            nc.scalar.activation(oflat[:, off:off + n], ps[:, 0:n], AF.Identity,
                                 bias=C['bc'][:], scale=1.0)
        nc.sync.dma_start(C['y'][:], out_sb[:])


def _wino_conv(C, src, n_cb, u_dram, bias_sb, dst_sbuf=None, dst_flat=None):
    """Winograd F(2x2,3x3) conv. src: 'x' | list(sbuf tiles) | ('mix', sbuf)."""
    nc, tc, t = C['nc'], C['tc'], C['t']
    with contextlib.ExitStack() as es:
        vp = es.enter_context(tc.tile_pool(name="wvp", bufs=1))
        tp = es.enter_context(tc.tile_pool(name="wtp", bufs=2))
        up = es.enter_context(tc.tile_pool(name="wup", bufs=4))
        zp = es.enter_context(tc.tile_pool(name="wzp", bufs=1))
        yp = es.enter_context(tc.tile_pool(name="wyp", bufs=2))
        xsp = es.enter_context(tc.tile_pool(name="wxp", bufs=3))
        psp = es.enter_context(tc.tile_pool(name="wps", bufs=2, space="PSUM"))
        V = [vp.tile([128, 8, 256], bf16, tag=f"v{i}", name=f"v{i}")
             for i in range(n_cb)]
        Z = [None] * 4

        def get_src(s, cb):
            if src == 'x' or (isinstance(src, tuple) and cb < 16):
                xs = xsp.tile([128, WLEN_W], bf16, tag="xs", name="xs")
                nc.sync.dma_start(xs[:, 0:WLEN],
                                  t['x'][cb][:, 16 * s * S:16 * s * S + WLEN])
                return xs, (lambda r, _s=s: r + 1 - 16 * _s)
            blk = src[1][cb - 16] if isinstance(src, tuple) else src[cb]
            return blk, (lambda r: r + 1)

        def vcompute_cb(s, jh, cb):
            """Emit T + V-column transform ops for one channel block."""
            eng = nc.vector
            sflat, bs = get_src(s, cb)
            T = tp.tile([128, 2, 8, 66], bf16, tag="T", name="T")
            for jl, (ra, rb, is_add) in enumerate(J_ROWS[jh]):
                op = eng.tensor_add if is_add else eng.tensor_sub
                op(T[:, jl], _band(sflat, bs, s, ra),
                   _band(sflat, bs, s, rb))
            for jl in range(2):
                pa = T[:, jl, :, 0:64].rearrange(
                    "q k (t two) -> q k t two", two=2)
                pb = T[:, jl, :, 2:66].rearrange(
                    "q k (t two) -> q k t two", two=2)
                e0, o1 = pa[:, :, :, 0], pa[:, :, :, 1]
                e2, o3 = pb[:, :, :, 0], pb[:, :, :, 1]
                for u, (ia, ib, add) in enumerate(
                        [(e0, e2, False), (o1, e2, True),
                         (e2, o1, False), (o1, o3, False)]):
                    dst = V[cb][:, 4 * jl + u, :].rearrange(
                        "q (k t) -> q k t", t=32)
                    (eng.tensor_add if add else eng.tensor_sub)(
                        dst, ia, ib)

        phases = [(s, jh) for s in range(4) for jh in range(2)]
        for pi, (s, jh) in enumerate(phases):
            if pi == 0:
                for cb in range(n_cb):
                    vcompute_cb(s, jh, cb)
            # ---- GEMMs + inverse; next phase's V interleaved into co=3
            for co in range(4):
                ps = psp.tile([128, 2, 4, 256], f32, tag="wps", name="wps")
                for cb in range(n_cb):
                    uw = up.tile([128, 8, 128], bf16, tag="uw", name="uw")
                    nc.sync.dma_start(uw[:], u_dram[jh, co, cb])
                    for jl in range(2):
                        for u in range(4):
                            # start only on the first group of each 2KB PSUM
                            # bank: the start bit zeroes the whole bank, and
                            # the second 1KB group lands on pending-zero.
                            nc.tensor.matmul(
                                ps[:, jl, u, :], uw[:, 4 * jl + u, :],
                                V[cb][:, 4 * jl + u, :],
                                start=(cb == 0 and u % 2 == 0),
                                stop=(cb == n_cb - 1),
                                skip_group_check=True)
                    if co == 3 and pi + 1 < len(phases):
                        vcompute_cb(*phases[pi + 1], cb)
                    if jh == 0:
                        Z[co] = zp.tile([128, 2, 4, 256], bf16, tag=f"z{co}",
                                        name=f"z{co}")
                        nc.vector.tensor_copy(Z[co][:, 1], ps[:, 1])
                        nc.vector.tensor_add(Z[co][:, 0], ps[:, 0], Z[co][:, 1])
                        continue
                    nc.vector.tensor_add(Z[co][:, 0], Z[co][:, 0], ps[:, 0])
                    nc.vector.tensor_sub(Z[co][:, 1], Z[co][:, 1], ps[:, 0])
                    nc.vector.tensor_sub(Z[co][:, 1], Z[co][:, 1], ps[:, 1])
                    Y = yp.tile([128, 2, 2, 256], f32, tag="y", name="ywino")
                    for p in range(2):
                        nc.vector.tensor_add(Y[:, p, 0], Z[co][:, p, 0],
                                             Z[co][:, p, 1])
                        nc.vector.tensor_add(Y[:, p, 0], Y[:, p, 0],
                                             Z[co][:, p, 2])
                        nc.vector.tensor_sub(Y[:, p, 1], Z[co][:, p, 1],
                                             Z[co][:, p, 2])
                        nc.vector.tensor_sub(Y[:, p, 1], Y[:, p, 1],
                                             Z[co][:, p, 3])
                    for p in range(2):
                        for i in range(2):
                            ysrc = Y[:, p, i, :].rearrange(
                                "q (k t) -> q k t", t=32)
                            if dst_sbuf is not None:
                                off0 = (16 * s + p + 1) * S + i + 1
                                dv = dst_sbuf[co][:, off0:off0 + 16 * S]
                                dv = dv.rearrange("q (k c) -> q k c", c=2 * S)
                                dv = dv[:, :, 0:64].rearrange(
                                    "q k (t two) -> q k t two", two=2)[..., 0]
                            else:
                                dv = dst_flat[co][:, 16 * s:16 * s + 16, :]
                                dv = dv.rearrange(
                                    "q (k two) (t tw) -> q k two t tw",
                                    two=2, tw=2)[:, :, p, :, i]
                            nc.scalar.activation(dv, ysrc, AF.Relu,
                                                 bias=bias_sb[co][:], scale=1.0)


def _cca(C, src_in, src_out):
    """Criss-cross attention: src_out = gamma*(outh+outw+v_b) + src_in."""
    nc, tc = C['nc'], C['tc']
    gamma, ident, identf = C['gamma'], C['ident'], C['identf']
    with contextlib.ExitStack() as es:
        atp = es.enter_context(tc.tile_pool(name="atp", bufs=1))
        ATh = atp.tile([64, 64, 64], bf16, tag="ATh")   # [j, w, h]
        ATw = atp.tile([64, 64, 64], bf16, tag="ATw")   # [j, h, w]
        eap = es.enter_context(tc.tile_pool(name="eap", bufs=1))
        EH = eap.tile([64, 64, 64], f32, tag="EH")      # [h, w, j]
        EW = eap.tile([64, 64, 64], f32, tag="EW")      # [w, h, j]
        RSH = eap.tile([64, 64], f32, tag="RSH")
        RSW = eap.tile([64, 64], f32, tag="RSW")
        Ssm = eap.tile([64, 64], f32, tag="Ssm")
        RIh = eap.tile([64, 64], f32, tag="RIh")
        RIw = eap.tile([64, 64], f32, tag="RIw")
        vtp = es.enter_context(tc.tile_pool(name="vtp", bufs=8))
        psV = es.enter_context(tc.tile_pool(name="psV", bufs=2, space="PSUM"))

        def vt_w_chunk(wc):
            VT = vtp.tile([64, 4, 512], bf16, tag="VT")
            for i in range(4):
                w = wc * 4 + i
                ps = psV.tile([64, 512], f32, tag="vps")
                for cb in range(4):
                    nc.tensor.matmul(ps[:], _rows(src_in[cb])[:, 1:65, w],
                                     C['wv'][cb][:],
                                     start=(cb == 0), stop=(cb == 3))
                (nc.scalar.activation if i % 2 else nc.vector.tensor_copy)(
                    *((VT[:, i, :], ps[:], AF.Copy) if i % 2
                      else (VT[:, i, :], ps[:])))
            return VT

        def vt_h_chunk(hc):
            VT = vtp.tile([64, 4, 512], bf16, tag="VT")
            for i in range(4):
                h = hc * 4 + i
                ps = psV.tile([64, 512], f32, tag="vps")
                for cb in range(4):
                    nc.tensor.matmul(ps[:], _rows(src_in[cb])[:, h + 1, 0:64],
                                     C['wv'][cb][:],
                                     start=(cb == 0), stop=(cb == 3))
                (nc.scalar.activation if i % 2 else nc.vector.tensor_copy)(
                    *((VT[:, i, :], ps[:], AF.Copy) if i % 2
                      else (VT[:, i, :], ps[:])))
            return VT

        # ---- phase A: q/k convs + energies
        with contextlib.ExitStack() as esA:
            qkp = esA.enter_context(tc.tile_pool(name="qkp", bufs=1))
            psQ = esA.enter_context(tc.tile_pool(name="psQ", bufs=2, space="PSUM"))
            psE = esA.enter_context(tc.tile_pool(name="psE", bufs=4, space="PSUM"))
            q_sb = qkp.tile([64, 64, 65], bf16, tag="q")
            k_sb = qkp.tile([64, 64, 65], bf16, tag="k")
            for dst_sb, wgt, bias in [(q_sb, C['wq'], C['bq']),
                                      (k_sb, C['wk'], C['bk'])]:
                dflat = dst_sb[:].rearrange("p r c -> p (r c)")
                for off, n in QK_TILES:
                    ps = psQ.tile([64, 512], f32, tag="qkps")
                    for cb in range(4):
                        rhs = src_in[cb][:, IMG0 + off:IMG0 + off + n]
                        nc.tensor.matmul(ps[:, 0:n], wgt[cb][:], rhs,
                                         start=(cb == 0), stop=(cb == 3))
                    nc.scalar.activation(dflat[:, off:off + n], ps[:, 0:n],
                                         AF.Identity, bias=bias[:], scale=1.0)
            for wi in range(16):
                ps = psE.tile([64, 4, 64], f32, tag="e4")
                for k in range(4):
                    w = wi * 4 + k
                    nc.tensor.matmul(ps[:, k, :], q_sb[:, :, w], k_sb[:, :, w],
                                     start=True, stop=True)
                nc.vector.tensor_add(EH[:, wi * 4:wi * 4 + 4, :], ps[:],
                                     C['mask4'][:])
            for hi in range(16):
                ps = psE.tile([64, 4, 64], f32, tag="e4")
                for k in range(4):
                    h = hi * 4 + k
                    nc.tensor.matmul(ps[:, k, :], q_sb[:, h, 0:64],
                                     k_sb[:, h, 0:64], start=True, stop=True)
                nc.vector.tensor_copy(EW[:, hi * 4:hi * 4 + 4, :], ps[:])

        # ---- early VT (w-orientation) chunks 0..7: keeps PE busy in softmax
        vt_cache = {wc: vt_w_chunk(wc) for wc in range(8)}

        # ---- softmax (batched) + transposes
        with contextlib.ExitStack() as esS:
            ebp = esS.enter_context(tc.tile_pool(name="ebp", bufs=2))
            psS = esS.enter_context(tc.tile_pool(name="psS", bufs=2, space="PSUM"))
            psT = esS.enter_context(tc.tile_pool(name="psT", bufs=2, space="PSUM"))
            ehf = EH[:].rearrange("p a b -> p (a b)")
            ewf = EW[:].rearrange("p a b -> p (a b)")
            nc.scalar.activation(ehf, ehf, AF.Exp)
            nc.scalar.activation(ewf, ewf, AF.Exp)
            nc.vector.reduce_sum(RSH[:], EH[:], axis=AX.X)
            nc.vector.reduce_sum(RSW[:], EW[:], axis=AX.X)
            pst = psS.tile([64, 64], f32, tag="trS")
            nc.tensor.transpose(pst[:], RSW[:], identf[:])
            nc.vector.tensor_add(Ssm[:], RSH[:], pst[:])
            nc.vector.reciprocal(RIh[:], Ssm[:])
            pst2 = psS.tile([64, 64], f32, tag="trS")
            nc.tensor.transpose(pst2[:], Ssm[:], identf[:])
            nc.vector.reciprocal(RIw[:], pst2[:])
            for wi in range(16):
                eb = ebp.tile([64, 4, 64], bf16, tag="eb")
                pt = psT.tile([64, 4, 64], bf16, tag="at")
                for k in range(4):
                    w = wi * 4 + k
                    nc.scalar.activation(eb[:, k, :], EH[:, w, :], AF.Copy,
                                         scale=RIh[:, w:w + 1])
                    nc.tensor.transpose(pt[:, k, :], eb[:, k, :], ident[:])
                nc.scalar.activation(ATh[:, wi * 4:wi * 4 + 4, :], pt[:], AF.Copy)
            for hi in range(16):
                eb = ebp.tile([64, 4, 64], bf16, tag="eb")
                pt = psT.tile([64, 4, 64], bf16, tag="at")
                for k in range(4):
                    h = hi * 4 + k
                    nc.scalar.activation(eb[:, k, :], EW[:, h, :], AF.Copy,
                                         scale=RIw[:, h:h + 1])
                    nc.tensor.transpose(pt[:, k, :], eb[:, k, :], ident[:])
                nc.scalar.activation(ATw[:, hi * 4:hi * 4 + 4, :], pt[:], AF.Copy)

        # ---- phase C
        with contextlib.ExitStack() as esC:
            sgp = esC.enter_context(tc.tile_pool(name="sgp", bufs=3))
            psD = esC.enter_context(tc.tile_pool(name="psD", bufs=6, space="PSUM"))
            # w-phase: src_out = src_in + gamma*out_h
            for wc in range(16):
                VT = vt_cache.pop(wc) if wc in vt_cache else vt_w_chunk(wc)
                for cb in range(4):
                    pso = psD.tile([128, 4, 64], f32, tag="ops")
                    for i in range(4):
                        w = wc * 4 + i
                        nc.tensor.matmul(
                            pso[:, i, :], VT[:, i, cb * 128:(cb + 1) * 128],
                            ATh[:, w, :], start=True, stop=True)
                    stg = sgp.tile([128, 4, 64], bf16, tag="stg")
                    nc.scalar.activation(stg[:], pso[:], AF.Copy, scale=gamma)
                    o_sl = _rows(src_out[cb])[:, 1:65, wc * 4:wc * 4 + 4]
                    i_sl = _rows(src_in[cb])[:, 1:65, wc * 4:wc * 4 + 4]
                    nc.vector.tensor_add(o_sl, i_sl,
                                         stg[:].rearrange("p w h -> p h w"))
            # h-phase: src_out += gamma*out_w + gamma*v_b
            for hc in range(16):
                VT = vt_h_chunk(hc)
                for cb in range(4):
                    pso = psD.tile([128, 4, 64], f32, tag="ops")
                    for i in range(4):
                        h = hc * 4 + i
                        nc.tensor.matmul(
                            pso[:, i, :], VT[:, i, cb * 128:(cb + 1) * 128],
                            ATw[:, h, :], start=True, stop=True)
                    stg = sgp.tile([128, 4, 64], bf16, tag="stg")
                    nc.scalar.activation(stg[:], pso[:], AF.Identity,
                                         scale=gamma, bias=C['gvb'][cb][:])
                    o_sl = _rows(src_out[cb])[:, 1 + hc * 4:1 + hc * 4 + 4, 0:64]
                    nc.vector.tensor_add(o_sl, o_sl, stg[:])


_BUILD_CACHE = {}


def _get_nc(gamma):
    key = round(float(gamma), 12)
    if key not in _BUILD_CACHE:
        _BUILD_CACHE[key] = build(gamma, n_reps=1)
    return _BUILD_CACHE[key]


def kernel(**inputs):
    from concourse.bass_utils import run_bass_kernel_spmd
    inputs_np = {k: np.asarray(v) for k, v in inputs.items()}
    dev, gamma = host_prep(inputs_np)
    nc = _get_nc(gamma)
    in_maps = []
    for core in range(8):
        m = dict(dev)
        m['x'] = pad_x_host(np.asarray(inputs_np['x'][core], np.float32))
        in_maps.append(m)
    res = run_bass_kernel_spmd(nc, in_maps, core_ids=list(range(8)))
    out = np.stack([r['y'].reshape(104, 64, 64) for r in res.results])
    return out.astype(np.float32)
